# revision 36
# baseline (speedup 1.0000x reference)
"""Multi-head self-attention (RoPE, causal) on 8 Trainium2 NeuronCores.

Sharding: tensor-parallel over heads - 2 of 16 heads per core. Each core
computes its heads' Q/K/V projections, causal attention in a transposed
[dim, seq] layout, and a partial output projection against its row-slice
of w_o. The host sums the 8 partial outputs.

v2 schedule: single q/k projection with the RoPE partner obtained via a
128x128 permutation matmul (instead of a second full projection); scores
for two key-chunks merged per PSUM tile so one exp instruction covers
1024 columns; causal masking applied as a -1e30 bias add on PSUM before
exp (vector engine) so exp yields exact zeros; PV runs as an fp8e4m3
DoubleRow matmul (2x rate, 256-deep contraction per instruction) against
an augmented V carrying a ones-row for the softmax denominator; the
output projection is deferred and drip-fed into later attention steps;
projection work for step st+1 is interleaved into attention step st to
keep the tensor engine continuously busy while the activation engine
drains exps.
"""

import sys

sys.path.insert(0, "/opt/trn_rl_repo")

import numpy as np
import ml_dtypes

import concourse.bass as bass
import concourse.tile as tile

# ---------------------------------------------------------------------------
# Toolchain fixes (inlined, self-contained): walrus on this stack allows only
# one sync-wait per instruction; the Tile tail drain carries many.  Also the
# image's antenv lacks the NTFF profile hook.
# ---------------------------------------------------------------------------
from concourse.vector_clock import ScopedClock

MAXW = 1


def _patched_drain_and_barrier(self, tick_clock, wait_clock):
    nc = self.nc
    drain_inst = nc.sync.drain()
    wait_clock.add_sem_waits(
        drain_inst.ins, ScopedClock({None: tick_clock.global_clock})
    )
    si = drain_inst.ins.sync_info
    waits = list(si.on_wait or []) if si is not None else []
    if len(waits) > MAXW:
        si.on_wait = waits[:MAXW]
        rest = waits[MAXW:]
        while rest:
            chunk, rest = rest[:MAXW], rest[MAXW:]
            nop = nc.sync.nop(nofuse=True)
            nsi = nop.ins.sync_info
            if nsi is None:
                import bass_rust

                nop.ins.sync_info = bass_rust.SyncInfo(on_wait=chunk, on_update=[])
            else:
                nsi.on_wait = list(nsi.on_wait or []) + chunk

    nc.all_engine_barrier()
    assert self.sems is not None
    popped = nc._tile_sem_poison_stack.pop()
    assert popped is self._sem_poison
    nc.clear_and_free_semaphores(list(self.sems.allocated().values()))
    nc.all_engine_barrier()


def apply():
    tile.TileContext._drain_and_barrier = _patched_drain_and_barrier
    _install_ntff_hook_shim()
    _install_compile_hook()


def _split_waits_json(bir_json: bytes) -> bytes:
    """Walrus on this toolchain allows at most one sync-wait per instruction.
    Insert a same-engine NoOp carrying each excess wait immediately before any
    multi-wait instruction (engine blocks at the NoOp instead - identical
    semantics, order preserved)."""
    import json as _json

    d = _json.loads(bir_json)
    n_split = 0
    for fn in d.get("functions", []):
        for bb in fn.get("blocks", []):
            insts = bb.get("instructions", [])
            out = []
            for inst in insts:
                si = inst.get("sync_info")
                waits = (si or {}).get("on_wait") or []
                if len(waits) > 1:
                    ge = [w for w in waits if w.get("wait_mode") == "sem-ge-imm"]
                    other = [w for w in waits if w.get("wait_mode") != "sem-ge-imm"]
                    # keep one wait on the instruction (prefer a non-ge if present)
                    if other:
                        keep = other
                        move = ge
                    else:
                        keep = ge[-1:]
                        move = ge[:-1]
                    if len(keep) <= 1 and move:
                        for i, w in enumerate(move):
                            out.append(
                                {
                                    "debug": inst.get("debug", 0),
                                    "engine": inst["engine"],
                                    "ins": [],
                                    "outs": [],
                                    "name": f"{inst['name']}-ws{i}",
                                    "opcode": "NoOp",
                                    "sync_info": {"on_update": [], "on_wait": [w]},
                                }
                            )
                            n_split += 1
                        si["on_wait"] = keep
                out.append(inst)
            bb["instructions"] = out
    if n_split:
        print(f"tilefix: split {n_split} excess waits onto NoOps")
    return _json.dumps(d).encode()


def _install_compile_hook():
    import concourse.bass_utils as bu
    import concourse.bass2jax as b2j

    if getattr(bu, "_tilefix_wrapped", False):
        return
    orig = bu.compile_bir_kernel

    def wrapped(bir_json, tmpdir, neff_name="file.neff"):
        return orig(_split_waits_json(bir_json), tmpdir, neff_name)

    bu.compile_bir_kernel = wrapped
    b2j.compile_bir_kernel = wrapped
    bu._tilefix_wrapped = True


def _install_ntff_hook_shim():
    """The image's antenv package lacks axon_hooks; provide a stand-in module
    exposing the ctypes-based NTFF profile hook against /opt/axon/libaxon_pjrt.so
    so run_bass_kernel_spmd(trace=True) works."""
    import sys as _sys
    import types

    if "antenv.axon_hooks" in _sys.modules:
        return
    mod = types.ModuleType("antenv.axon_hooks")
    _state = {"hook": None}

    so_path = "/opt/axon/libaxon_pjrt.so"
    try:
        import importlib.util

        spec = importlib.util.spec_from_file_location(
            "trn_agent_boot.trn_boot", "/root/.axon_site/trn_agent_boot/trn_boot.py"
        )
        import trn_agent_boot.trn_boot as _tb  # type: ignore

        _state["hook"] = _tb._ntff_profile_via_ctypes(so_path)
    except Exception:
        _state["hook"] = None

    def get_axon_ntff_profile_hook():
        return _state["hook"]

    def set_axon_ntff_profile_hook(h):
        _state["hook"] = h

    mod.get_axon_ntff_profile_hook = get_axon_ntff_profile_hook
    mod.set_axon_ntff_profile_hook = set_axon_ntff_profile_hook
    _sys.modules["antenv.axon_hooks"] = mod


apply()

from concourse import mybir
from concourse.bass_utils import run_bass_kernel_spmd
from concourse.masks import make_identity

F32 = mybir.dt.float32
F32R = mybir.dt.float32r
BF16 = mybir.dt.bfloat16
FP16 = mybir.dt.float16
F8 = mybir.dt.float8e4
EXP = mybir.ActivationFunctionType.Exp
DR = mybir.MatmulPerfMode.DoubleRow

S = 4096          # sequence length
D = 1024          # model dim
NH = 16           # heads
HD = 64           # head dim
NCORES = 8
HPC = NH // NCORES  # heads per core = 2
QT = 512          # qpos tile (free dim of S^T / PV matmuls)
KC = 128          # kpos chunk (partition dim of S^T tiles)
NQT = S // QT     # 8
NKC = S // KC     # 32
NPAIR = NKC // 2  # 16 key-chunk pairs
NEG = -1.0e30

USE_FP8_PV = False       # fp8 PV measured at 2.5e-2 full-output rel err
                         # (gate is 2e-2) - bf16 PV gives 1.7e-3
DIRECT_PSUM_DMA = False  # bass dma_start cannot read PSUM
MASK_ON_POOL = True      # causal zeroing via gpsimd affine_select on pt
                         # (post-exp, SBUF) instead of DVE -1e30 adds on PSUM
DEFER_TAIL = False       # issue step qt's normalize tail inside step qt+1's
                         # pair stream (False = at the end of step qt);
                         # True raced: outproj drains could read attnT
                         # before the deferred tail wrote it
DVE_EXP_FRAC = True      # offload ~1/4 of off-diagonal exps to the DVE via
                         # the bf16 bit-trick (scalar engine is the pacer in
                         # late attention steps)
# bf16-bit Schraudolph constants: bits = x*0.125*(128/ln2) + 127*128
EXPA = 0.125 * 128.0 / float(np.log(2.0))
EXPB = 127.0 * 128.0 - 2.0 * 128.0 / float(np.log(2.0))
PV_ROWSPLIT = False      # split PV contraction into two 64-row-group matmuls
                         # (tile_position rows 0/64) that run concurrently;
                         # concurrent accumulation into the same PSUM
                         # addresses - suspected runtime hang, keep off
# NOTE: this walrus rejects every bass_isa extended op (gpsimd
# partition_broadcast, custom-DVE reciprocal_approx_*) with "ISA wrong
# length", so the tail keeps the plain DVE reciprocal + f32r ones-matmul
# broadcast; the two heads share one PSUM pb tile via column groups.

_CACHE = {}


def _build_nc():
    nc = bass.Bass("TRN2")

    xT_d = nc.dram_tensor("xT", [D, S], BF16, kind="ExternalInput")
    wq_d = nc.dram_tensor("wq", [D, 128], BF16, kind="ExternalInput")
    wk_d = nc.dram_tensor("wk", [D, 128], BF16, kind="ExternalInput")
    wv_d = nc.dram_tensor("wv", [D, 128], BF16, kind="ExternalInput")
    wo_d = nc.dram_tensor("wo", [128, D], BF16, kind="ExternalInput")
    cos_d = nc.dram_tensor("cosP", [128, S], FP16, kind="ExternalInput")
    sin_d = nc.dram_tensor("sinPs", [128, S], FP16, kind="ExternalInput")
    swp_d = nc.dram_tensor("swapP", [128, 128], BF16, kind="ExternalInput")
    neg_d = nc.dram_tensor("negm", [128, 256], F32, kind="ExternalInput")
    out_d = nc.dram_tensor("out", [S, D], F32, kind="ExternalOutput")

    VDT = F8 if USE_FP8_PV else BF16

    with tile.TileContext(nc) as tc:
        with (
            tc.tile_pool(name="const", bufs=1) as cpool,
            tc.tile_pool(name="big", bufs=1) as big,
            tc.tile_pool(name="xp", bufs=3) as xp,
            tc.tile_pool(name="wk", bufs=6) as wkp,
            tc.tile_pool(name="pt", bufs=8) as ptp,
            tc.tile_pool(name="rc", bufs=4) as rcp_pool,
            tc.tile_pool(name="ob", bufs=4) as obp,
            tc.tile_pool(name="pa", bufs=2, space="PSUM") as pa,
            tc.tile_pool(name="pss", bufs=2, space="PSUM") as pss,
            tc.tile_pool(name="pvh", bufs=2, space="PSUM") as pvh,
        ):
            # ---- minimal consts for proj(0): its matmuls wait only on the
            # projection weights + x chunk DMAs, nothing else ---------------
            identb = cpool.tile([128, 128], BF16)
            make_identity(nc, identb[:])
            w_sb = {}
            for name, dram in (("q", wq_d), ("k", wk_d), ("v", wv_d)):
                t = cpool.tile([128, 8, 128], BF16, name=f"w_{name}")
                nc.sync.dma_start(
                    t[:], dram.ap().rearrange("(kc p) m -> p kc m", p=128)
                )
                w_sb[name] = t
            swap_sb = cpool.tile([128, 128], BF16)
            nc.sync.dma_start(swap_sb[:], swp_d.ap())
            cosP = cpool.tile([128, S], FP16)
            sinPs = cpool.tile([128, S], FP16)
            if not MASK_ON_POOL:
                negm = cpool.tile([128, 256], F32)
            ones1 = cpool.tile([1, 64], F32R)
            expbias = cpool.tile([128, 1], F32)
            wo_sb = cpool.tile([128, D], BF16)

            def late_consts():
                # chunked so step-0 rope only waits on its own slice
                for c8 in range(8):
                    csl = slice(c8 * QT, (c8 + 1) * QT)
                    nc.sync.dma_start(cosP[:, csl], cos_d.ap()[:, csl])
                    nc.sync.dma_start(sinPs[:, csl], sin_d.ap()[:, csl])
                if not MASK_ON_POOL:
                    nc.sync.dma_start(negm[:], neg_d.ap())
                # -1 row: the Newton reciprocal lands as -1/denom, the
                # broadcast matmul against -1 restores the sign
                nc.scalar.activation(
                    ones1[:], cosP[0:1, 0:64],
                    mybir.ActivationFunctionType.Copy, bias=-1.0, scale=0.0,
                )
                nc.vector.memset(expbias[:], -2.0)
                nc.sync.dma_start(wo_sb[:], wo_d.ap())

            # ---- persistent activations -----------------------------------
            qT = big.tile([128, S], BF16)   # rope'd q, [2*64 dims, seq]
            kT = big.tile([128, S], BF16)
            attnT = big.tile([128, S], BF16)  # normalized attn out, [dims, seq]
            # augmented V: [kpos, pair, slot, head, 64 dims + ones col]
            vaug = big.tile([128, NPAIR, 2, HPC, 65], VDT, name="vaug")

            def init_vaug():
                va_flat = vaug[:].rearrange("p a b c o -> p (a b c) o")
                nc.scalar.activation(
                    va_flat[:, :, 64:65], identb[:, 0:NPAIR * 2 * HPC],
                    mybir.ActivationFunctionType.Copy, bias=1.0, scale=0.0,
                )

            xblk = {}

            # ---- projection units (one step = dma + q + k + v) ------------
            def dma_unit(st):
                sl = slice(st * QT, (st + 1) * QT)
                xb = xp.tile([128, 8, QT], BF16, tag="xblk", name=f"xb{st}")
                for kc8 in range(8):
                    nc.sync.dma_start(
                        xb[:, kc8, :],
                        xT_d.ap()[kc8 * 128:(kc8 + 1) * 128, sl],
                    )
                xblk[st] = xb

            def qk_unit(st, which):
                sl = slice(st * QT, (st + 1) * QT)
                dst = qT if which == "q" else kT
                xb = xblk[st]
                pa_t = pa.tile([128, QT], F32, tag="pa")
                for kc8 in range(8):
                    nc.tensor.matmul(
                        pa_t[:], w_sb[which][:, kc8, :], xb[:, kc8, :],
                        start=(kc8 == 0), stop=(kc8 == 7),
                    )
                qsb = wkp.tile([128, QT], BF16, tag="qsb")
                nc.vector.tensor_copy(qsb[:], pa_t[:])
                ps2 = pa.tile([128, QT], F32, tag="pa")
                nc.tensor.matmul(ps2[:], swap_sb[:], qsb[:],
                                 start=True, stop=True)
                t1 = wkp.tile([128, QT], BF16, tag="t1")
                nc.vector.tensor_mul(t1[:], ps2[:], sinPs[:, sl])
                t2 = wkp.tile([128, QT], BF16, tag="t2")
                nc.vector.tensor_mul(t2[:], qsb[:], cosP[:, sl])
                nc.vector.tensor_add(dst[:, sl], t1[:], t2[:])

            def v_unit(st):
                xb = xblk[st]
                pa_t = pa.tile([128, QT], F32, tag="pa")
                for kc8 in range(8):
                    nc.tensor.matmul(
                        pa_t[:], w_sb["v"][:, kc8, :], xb[:, kc8, :],
                        start=(kc8 == 0), stop=(kc8 == 7),
                    )
                vtmp = wkp.tile([128, QT], BF16, tag="vtmp")
                nc.vector.tensor_copy(vtmp[:], pa_t[:])
                for z in range(4):
                    kc = st * 4 + z
                    pr, sl8 = kc // 2, kc % 2
                    pst = pa.tile([128, 128], BF16, tag="pa")
                    nc.tensor.transpose(
                        pst[:], vtmp[:, z * 128:(z + 1) * 128], identb[:]
                    )
                    pst3 = pst[:].rearrange("p (h d) -> p h d", h=HPC)
                    nc.vector.tensor_copy(vaug[:, pr, sl8, :, 0:64], pst3)

            def proj_units(st):
                return [
                    lambda st=st: dma_unit(st),
                    lambda st=st: qk_unit(st, "q"),
                    lambda st=st: qk_unit(st, "k"),
                    lambda st=st: v_unit(st),
                ]

            # ---- attention -------------------------------------------------
            def make_po(qt):
                return [
                    pvh.tile([128, QT], F32, tag="pvh", name=f"po{qt}_{h}")
                    for h in range(HPC)
                ]

            def pair_scores_exp(qt, p, po_h):
                """Scores (2 chunks x 2 heads) + causal mask + exp -> pt tiles.
                Returns the two pt tiles (one per head) for the deferred PV.

                The two heads' score matmuls are issued adjacently: h0's
                stationary/moving operands sit at partitions 0-63 and h1's at
                64-127, so they land in disjoint PE row-groups (auto
                tile_position) and execute concurrently."""
                pairjoff = max(0, (2 * p - 4 * qt)) * 128
                ps_ts = [
                    pss.tile([128, 2, QT], F32, tag="pss",
                             name=f"ss{qt}_{p}_{h}")
                    for h in range(HPC)
                ]
                for s8 in range(2):
                    kc = 2 * p + s8
                    for h in range(HPC):
                        hsl = slice(h * 64, (h + 1) * 64)
                        # write from pairjoff so the pair-wide exp below
                        # never reads uninitialized PSUM (slot 1's columns
                        # below its own causal offset are computed but then
                        # excluded from PV, which reads from joff onward)
                        nc.tensor.matmul(
                            ps_ts[h][:, s8, pairjoff:QT],
                            kT[hsl, kc * 128:(kc + 1) * 128],
                            qT[hsl, qt * QT + pairjoff:(qt + 1) * QT],
                            start=True, stop=True,
                        )
                pts = []
                for h in range(HPC):
                    ps_t = ps_ts[h]
                    diag = p >= 2 * qt
                    if diag and not MASK_ON_POOL:
                        if p == 2 * qt:
                            nc.vector.tensor_add(
                                ps_t[:, 0, 0:128], ps_t[:, 0, 0:128],
                                negm[:, 128:256])
                            nc.vector.tensor_add(
                                ps_t[:, 1, 0:256], ps_t[:, 1, 0:256],
                                negm[:, 0:256])
                        else:
                            nc.vector.tensor_add(
                                ps_t[:, 0, 256:384], ps_t[:, 0, 256:384],
                                negm[:, 128:256])
                            nc.vector.tensor_add(
                                ps_t[:, 1, 256:512], ps_t[:, 1, 256:512],
                                negm[:, 0:256])
                    pt_t = ptp.tile([128, 2, QT], VDT, tag="pt",
                                    name=f"pt{qt}_{p}_{h}")
                    # bias shifts all probs by e^-2 (cancels in the rowsum
                    # normalization) so fp8's max never overflows on tail
                    # scores; fp8 is floating point, so relative resolution
                    # of the probs is unchanged
                    if DVE_EXP_FRAC and not diag and (p % 4 == 1):
                        # bit-trick exp on the DVE for ~1/4 of the
                        # off-diagonal pairs: bf16(exp(x)) bits equal
                        # round(x*128/ln2 + 127*128) in int16 up to the
                        # linear-mantissa sawtooth (<6% per prob, cancels
                        # to ~1e-3 on the output through the softmax);
                        # offloads the saturated scalar engine
                        nc.vector.tensor_scalar(
                            out=pt_t[:, :, pairjoff:QT].bitcast(
                                mybir.dt.int16),
                            in0=ps_t[:, :, pairjoff:QT],
                            scalar1=EXPA, scalar2=EXPB,
                            op0=mybir.AluOpType.mult,
                            op1=mybir.AluOpType.add)
                    else:
                        nc.scalar.activation(
                            pt_t[:, :, pairjoff:QT], ps_t[:, :, pairjoff:QT],
                            EXP, scale=0.125, bias=expbias[:],
                        )
                    if diag and MASK_ON_POOL:
                        # zero the 128-wide causal triangle of each chunk:
                        # keep qpos-col c >= kpos-row p within the window
                        # starting at the chunk's causal offset
                        for s8 in range(2):
                            joff = max(0, 2 * p + s8 - 4 * qt) * 128
                            nc.gpsimd.affine_select(
                                out=pt_t[:, s8, joff:joff + 128],
                                in_=pt_t[:, s8, joff:joff + 128],
                                compare_op=mybir.AluOpType.is_ge,
                                fill=0.0,
                                base=0,
                                pattern=[[1, 128]],
                                channel_multiplier=-1,
                            )
                    pts.append(pt_t)
                return pts

            def pair_pv(qt, p, po_h, pts):
                pairjoff = max(0, (2 * p - 4 * qt)) * 128
                last = (p == 2 * qt + 1)
                for h in range(HPC):
                    if USE_FP8_PV:
                        nc.tensor.matmul(
                            po_h[h][0:65, pairjoff:QT],
                            vaug[:, p, :, h, :],
                            pts[h][:, :, pairjoff:QT],
                            start=(p == 0), stop=last,
                            perf_mode=DR,
                        )
                    elif PV_ROWSPLIT:
                        # split the 128-deep contraction into two 64-row
                        # matmuls pinned to disjoint PE row-groups; they
                        # execute concurrently and accumulate into the same
                        # PSUM tile (drain order is pc order, so the
                        # start=True clear lands first)
                        for s8 in range(2):
                            kc = 2 * p + s8
                            joff = max(0, kc - 4 * qt) * 128
                            for rg in range(2):
                                nc.tensor.matmul(
                                    po_h[h][0:65, joff:QT],
                                    vaug[rg * 64:(rg + 1) * 64, p, s8, h, :],
                                    pts[h][rg * 64:(rg + 1) * 64, s8, joff:QT],
                                    start=(p == 0 and s8 == 0 and rg == 0),
                                    stop=(last and s8 == 1 and rg == 1),
                                    tile_position=(rg * 64, 0),
                                )
                    else:
                        for s8 in range(2):
                            kc = 2 * p + s8
                            joff = max(0, kc - 4 * qt) * 128
                            nc.tensor.matmul(
                                po_h[h][0:65, joff:QT],
                                vaug[:, p, s8, h, :],
                                pts[h][:, s8, joff:QT],
                                start=(p == 0 and s8 == 0), stop=(last and s8 == 1),
                            )

            C_RECIP = float(0x7EF311C3)

            def tail(qt, po_h):
                """Normalize step qt into attnT: -1/denominator via a
                bit-trick seed + one Newton pass (3 standard DVE ops, ~2x
                cheaper than the native iterative-divide RECIPROCAL and
                usable because this walrus rejects all custom-DVE ops),
                then the f32r ones(-1)-broadcast matmul and the normalize
                muls, pipelined per head (walrus rejects a dst-partition-64
                col-tiled matmul, so each head gets its own pb tile)."""
                qsl = slice(qt * QT, (qt + 1) * QT)
                for h in range(HPC):
                    pb = pa.tile([128, QT], F32, tag="pa", name=f"pb{qt}_{h}")
                    # PSUM reads are value-converted per the instruction
                    # dtype, so the bit trick must run on an SBUF copy
                    den = rcp_pool.tile([1, QT], F32, tag="rc",
                                        name=f"dn{qt}_{h}")
                    nc.vector.tensor_copy(den[:], po_h[h][64:65, :])
                    seed = rcp_pool.tile([1, QT], mybir.dt.int32, tag="rc",
                                         name=f"sd{qt}_{h}")
                    nc.vector.tensor_scalar(
                        out=seed[:], in0=den[:].bitcast(mybir.dt.int32),
                        scalar1=C_RECIP, scalar2=-1.0,
                        op0=mybir.AluOpType.subtract,
                        op1=mybir.AluOpType.mult)
                    t_nr = rcp_pool.tile([1, QT], F32, tag="rc",
                                         name=f"tn{qt}_{h}")
                    nc.vector.tensor_mul(t_nr[:], den[:], seed[:].bitcast(F32))
                    rc_t = rcp_pool.tile([1, QT], F32R, tag="rc",
                                         name=f"rc{qt}_{h}")
                    with nc.allow_low_precision(
                        reason="reciprocal at ~11 bits after one Newton "
                               "pass; probs are bf16 anyway"
                    ):
                        nc.vector.scalar_tensor_tensor(
                            out=rc_t[:], in0=t_nr[:], scalar=-2.0,
                            in1=seed[:].bitcast(F32),
                            op0=mybir.AluOpType.add,
                            op1=mybir.AluOpType.mult)
                    nc.tensor.matmul(pb[0:64, :], ones1[:],
                                     rc_t[:], start=True, stop=True)
                    # walrus allows at most one PSUM operand per tensor
                    # op, so bounce the broadcast through SBUF
                    rb = wkp.tile([64, QT], F32, tag="rb", name=f"rb{qt}_{h}")
                    nc.vector.tensor_copy(rb[:], pb[0:64, :])
                    hsl = slice(h * 64, (h + 1) * 64)
                    # 128-col chunks: each outproj unit reads a 128-col
                    # slice of attnT, so it can start as soon as its chunk
                    # of the normalize lands
                    for zc in range(4):
                        csl = slice(qt * QT + zc * 128, qt * QT + (zc + 1) * 128)
                        nc.vector.tensor_mul(
                            attnT[hsl, csl], po_h[h][0:64, zc * 128:(zc + 1) * 128],
                            rb[:, zc * 128:(zc + 1) * 128]
                        )

            def outproj_unit(qt, z, ncol):
                csl = slice(qt * QT + z * 128, qt * QT + (z + 1) * 128)
                osl = slice(ncol * 512, (ncol + 1) * 512)
                pout = pa.tile([128, QT], F32, tag="pa")
                nc.tensor.matmul(pout[:], attnT[:, csl], wo_sb[:, osl],
                                 start=True, stop=True)
                osb = obp.tile([128, QT], F32, tag="ob")
                # DVE does all PSUM->SBUF bounces: the scalar engine is the
                # co-bottleneck (exp stream), DVE has headroom once the
                # full-width reciprocals are gone
                nc.vector.tensor_copy(osb[:], pout[:])
                nc.sync.dma_start(out_d.ap()[csl, osl], osb[:])

            # ---- schedule --------------------------------------------------
            from collections import deque

            fill_q = deque()    # (deadline_step, unit_fn): projections
            deferred = deque()  # outproj units from earlier steps

            u_dma, u_q, u_k, u_v = proj_units(0)
            u_dma()
            late_consts()
            init_vaug()
            u_q(); u_k(); u_v()
            for st in range(1, NQT):
                for u in proj_units(st):
                    fill_q.append((st, u))

            pending_tail = None  # (qt, po_h) normalize deferred into next step
            for qt in range(NQT):
                npairs = 2 * qt + 2
                due = []
                while fill_q and fill_q[0][0] <= qt + 1:
                    due.append(fill_q.popleft()[1])
                po_h = make_po(qt)
                pending = None  # (p, pts) awaiting PV
                due_done = 0
                for p in range(npairs):
                    if DEFER_TAIL and pending_tail is not None and p == 1:
                        # issue the previous step's normalize tail here -
                        # after this step's first scores (PE keeps
                        # streaming over the reciprocal latency) but BEFORE
                        # pair_pv(p=0) below, whose PSUM buffers rotate
                        # onto the tiles the tail still reads, and before
                        # any drained outproj that reads the attnT range
                        # this tail writes
                        tail(*pending_tail)
                        pending_tail = None
                    target = -(-len(due) * (p + 1) // npairs)  # ceil
                    had_due = due_done < target
                    while due_done < target:
                        due[due_done]()
                        due_done += 1
                    # drain deferred outprojs when this pair brought no proj
                    # work; also drain at the first two pairs of each step to
                    # keep PE duty high through the normalize-tail boundary
                    # (the HAM re-throttles on window duty, not just full idle)
                    if deferred and (not had_due or p <= 1):
                        n_drain = 3 if qt >= 4 else (2 if qt >= 2 else 1)
                        if p <= 1:
                            n_drain = max(n_drain, 2)
                        for _ in range(n_drain):
                            if deferred:
                                deferred.popleft()()
                    if pending is not None:
                        pair_pv(qt, pending[0], po_h, pending[1])
                    pending = (p, pair_scores_exp(qt, p, po_h))
                pair_pv(qt, pending[0], po_h, pending[1])
                if DEFER_TAIL:
                    assert pending_tail is None
                    pending_tail = (qt, po_h)
                else:
                    tail(qt, po_h)
                for z in range(4):
                    for ncol in range(2):
                        deferred.append(
                            lambda qt=qt, z=z, ncol=ncol: outproj_unit(qt, z, ncol)
                        )
            if pending_tail is not None:
                tail(*pending_tail)
            while deferred:
                deferred.popleft()()

    return nc


def _rope_tables(token_positions):
    """cosP/sinPs in the transposed per-partition layout.

    Row r (r in 0..127): head = r//64, idx = r%64; pair j = idx%32.
    Rows with idx<32 hold even rope dims (d=2j), idx>=32 odd dims (d=2j+1).
    sinPs is the *swap-adjusted* sin table: the swapped projection holds the
    partner value (q_odd on even rows, q_even on odd rows), so even rows
    need -sin (r_e = q_e cos - q_o sin) and odd rows +sin (r_o = q_o cos +
    q_e sin).
    """
    pos = token_positions.astype(np.float32)  # [S]
    inv = (1.0 / (10000.0 ** (np.arange(0, HD, 2, dtype=np.float32) / HD)))
    freqs = pos[:, None] * inv[None, :]        # [S, 32]
    cos32 = np.cos(freqs).T.astype(np.float32)  # [32, S]
    sin32 = np.sin(freqs).T.astype(np.float32)
    cosP = np.concatenate([cos32, cos32, cos32, cos32], 0)
    sinPs = np.concatenate([-sin32, sin32, -sin32, sin32], 0)
    return (np.ascontiguousarray(cosP).astype(np.float16),
            np.ascontiguousarray(sinPs).astype(np.float16))


def kernel(x, w_q, w_k, w_v, w_o, token_positions):
    x = np.asarray(x, dtype=np.float32)
    w_q = np.asarray(w_q, dtype=np.float32)
    w_k = np.asarray(w_k, dtype=np.float32)
    w_v = np.asarray(w_v, dtype=np.float32)
    w_o = np.asarray(w_o, dtype=np.float32)
    tp = np.asarray(token_positions).reshape(-1)

    b = x.shape[0]
    assert x.shape == (b, S, D) and b == 1

    xT = np.ascontiguousarray(x[0].T).astype(ml_dtypes.bfloat16)  # [D, S]
    cosP, sinPs = _rope_tables(tp)

    # per-head permutation: evens (0,2,..62) then odds (1,3,..63)
    perm64 = np.concatenate([np.arange(0, HD, 2), np.arange(1, HD, 2)])
    # swap of the 32-blocks within each head: [32:64, 0:32] per 64-block
    swap128 = np.concatenate([
        np.arange(32, 64), np.arange(0, 32),
        np.arange(96, 128), np.arange(64, 96),
    ])
    swapP = np.eye(128, dtype=np.float32)[swap128].astype(ml_dtypes.bfloat16)
    negm = np.where(
        np.arange(256)[None, :] < 128 + np.arange(128)[:, None], NEG, 0.0
    ).astype(np.float32)

    if "nc" not in _CACHE:
        _CACHE["nc"] = _build_nc()
    nc = _CACHE["nc"]

    in_maps = []
    for c in range(NCORES):
        rows = np.concatenate(
            [c * 128 + h * 64 + perm64 for h in range(HPC)]
        )  # 128 permuted q/k output dims of this core
        in_maps.append({
            "xT": xT,
            "wq": np.ascontiguousarray(w_q[rows].T).astype(ml_dtypes.bfloat16),
            "wk": np.ascontiguousarray(w_k[rows].T).astype(ml_dtypes.bfloat16),
            "wv": np.ascontiguousarray(
                w_v[c * 128:(c + 1) * 128].T).astype(ml_dtypes.bfloat16),
            "wo": np.ascontiguousarray(
                w_o[:, c * 128:(c + 1) * 128].T).astype(ml_dtypes.bfloat16),
            "cosP": cosP,
            "sinPs": sinPs,
            "swapP": swapP,
            "negm": negm,
        })

    _CACHE["last_in_maps"] = in_maps
    res = run_bass_kernel_spmd(nc, in_maps, core_ids=list(range(NCORES)))
    _CACHE["last_res"] = res
    out = res.results[0]["out"].astype(np.float64)
    for c in range(1, NCORES):
        out += res.results[c]["out"]
    return out.astype(np.float32)[None]


if __name__ == "__main__":
    rng = np.random.default_rng(0)
    x = rng.standard_normal((1, S, D), dtype=np.float32)
    sc = 1.0 / np.sqrt(D)
    wq = rng.standard_normal((D, D), dtype=np.float32) * sc
    wk = rng.standard_normal((D, D), dtype=np.float32) * sc
    wv = rng.standard_normal((D, D), dtype=np.float32) * sc
    wo = rng.standard_normal((D, D), dtype=np.float32) * sc
    tpos = np.arange(S, dtype=np.int32)[None]
    out = kernel(x=x, w_q=wq, w_k=wk, w_v=wv, w_o=wo, token_positions=tpos)
    print("kernel out:", out.shape, out.dtype, float(np.abs(out).max()))
    if "last_res" in _CACHE:
        dbg = _CACHE["last_res"].results[0].get("dbg")
        if dbg is not None:
            names = ["den ", "seed", "t_nr", "rc  "]
            for r in range(4):
                row = dbg[r]
                print(f"dbg {names[r]}: min={np.nanmin(row):.4e} "
                      f"max={np.nanmax(row):.4e} nan={np.isnan(row).sum()}"
                      f" samples={row[[0, 5, 600, 2000, 4095]]}")
        # numpy reference for the per-core partial to localize corruption
        def np_partial_unused(c):
            rows_v = slice(c * 128, (c + 1) * 128)
            qh = x[0] @ w_q.T
            kh = x[0] @ w_k.T
            vh = (x[0] @ w_v.T)[:, rows_v]
            pos = np.arange(S, dtype=np.float32)
            inv = 1.0 / (10000.0 ** (np.arange(0, HD, 2, np.float32) / HD))
            fr = pos[:, None] * inv[None, :]
            cosn, sinn = np.cos(fr), np.sin(fr)
            def rope_np(t):
                t = t.reshape(S, NH, HD)
                e, o = t[:, :, 0::2], t[:, :, 1::2]
                re = e * cosn[:, None, :] - o * sinn[:, None, :]
                ro = e * sinn[:, None, :] + o * cosn[:, None, :]
                z = np.empty_like(t)
                z[:, :, 0::2] = re
                z[:, :, 1::2] = ro
                return z
            qr, kr = rope_np(qh), rope_np(kh)
            msk = np.tril(np.ones((S, S), bool))
            outp = np.zeros((S, D), np.float32)
            for hh in range(2 * c, 2 * c + 2):
                s = qr[:, hh] @ kr[:, hh].T / 8.0
                s = np.where(msk, s, -np.inf)
                p = np.exp(s - s.max(-1, keepdims=True))
                p /= p.sum(-1, keepdims=True)
                at = p @ vh[:, (hh - 2 * c) * 64:(hh - 2 * c + 1) * 64]
                outp += at @ w_o[:, hh * 64:(hh + 1) * 64].T
            return outp
    # race detector: rerun with cached NEFF, compare
    outs = [out]
    for rep in range(3):
        o2 = kernel(x=x, w_q=wq, w_k=wk, w_v=wv, w_o=wo, token_positions=tpos)
        d = float(np.nanmax(np.abs(o2 - outs[0])))
        print(f"rerun {rep}: max={float(np.abs(o2).max()):.4g} "
              f"nan={int(np.isnan(o2).sum())} diff-vs-run0={d:.4g}")
        outs.append(o2)


# revision 38
# speedup vs baseline: 1.0634x; 1.0634x over previous
"""Multi-head self-attention (RoPE, causal) on 8 Trainium2 NeuronCores.

Sharding: tensor-parallel over heads - 2 of 16 heads per core. Each core
computes its heads' Q/K/V projections, causal attention in a transposed
[dim, seq] layout, and a partial output projection against its row-slice
of w_o. The host sums the 8 partial outputs.

v3 schedule: single q/k projection with the RoPE partner obtained via a
128x128 permutation matmul; scores for two key-chunks merged per PSUM
tile so one exp instruction covers 1024 columns; the two heads' score
matmuls are issued adjacently so their disjoint PE row-groups (K=64 at
partitions 0:64 / 64:128) execute concurrently; causal zeroing via
gpsimd affine_select post-exp; PV in bf16 against an augmented V
carrying a ones-row for the softmax denominator; the normalize tail
uses a bit-seed + one-Newton reciprocal (3 standard DVE ops, the only
kind this walrus accepts) into an f32r ones(-1)-broadcast matmul, with
the normalize muls chunked 128 cols so output-projection units unblock
early; output projection is deferred and drip-fed into later attention
steps (with forced drains at the first two pairs of each step to hold
PE duty through the boundary); projection work for step st+1 is
interleaved into attention step st.
"""

import sys

sys.path.insert(0, "/opt/trn_rl_repo")

import numpy as np
import ml_dtypes

import concourse.bass as bass
import concourse.tile as tile

# ---------------------------------------------------------------------------
# Toolchain fixes (inlined, self-contained): walrus on this stack allows only
# one sync-wait per instruction; the Tile tail drain carries many.  Also the
# image's antenv lacks the NTFF profile hook.
# ---------------------------------------------------------------------------
from concourse.vector_clock import ScopedClock

MAXW = 1


def _patched_drain_and_barrier(self, tick_clock, wait_clock):
    nc = self.nc
    drain_inst = nc.sync.drain()
    wait_clock.add_sem_waits(
        drain_inst.ins, ScopedClock({None: tick_clock.global_clock})
    )
    si = drain_inst.ins.sync_info
    waits = list(si.on_wait or []) if si is not None else []
    if len(waits) > MAXW:
        si.on_wait = waits[:MAXW]
        rest = waits[MAXW:]
        while rest:
            chunk, rest = rest[:MAXW], rest[MAXW:]
            nop = nc.sync.nop(nofuse=True)
            nsi = nop.ins.sync_info
            if nsi is None:
                import bass_rust

                nop.ins.sync_info = bass_rust.SyncInfo(on_wait=chunk, on_update=[])
            else:
                nsi.on_wait = list(nsi.on_wait or []) + chunk

    nc.all_engine_barrier()
    assert self.sems is not None
    popped = nc._tile_sem_poison_stack.pop()
    assert popped is self._sem_poison
    nc.clear_and_free_semaphores(list(self.sems.allocated().values()))
    nc.all_engine_barrier()


def apply():
    tile.TileContext._drain_and_barrier = _patched_drain_and_barrier
    _install_ntff_hook_shim()
    _install_compile_hook()


def _split_waits_json(bir_json: bytes) -> bytes:
    """Walrus on this toolchain allows at most one sync-wait per instruction.
    Insert a same-engine NoOp carrying each excess wait immediately before any
    multi-wait instruction (engine blocks at the NoOp instead - identical
    semantics, order preserved)."""
    import json as _json

    d = _json.loads(bir_json)
    n_split = 0
    for fn in d.get("functions", []):
        for bb in fn.get("blocks", []):
            insts = bb.get("instructions", [])
            out = []
            for inst in insts:
                si = inst.get("sync_info")
                waits = (si or {}).get("on_wait") or []
                if len(waits) > 1:
                    ge = [w for w in waits if w.get("wait_mode") == "sem-ge-imm"]
                    other = [w for w in waits if w.get("wait_mode") != "sem-ge-imm"]
                    # keep one wait on the instruction (prefer a non-ge if present)
                    if other:
                        keep = other
                        move = ge
                    else:
                        keep = ge[-1:]
                        move = ge[:-1]
                    if len(keep) <= 1 and move:
                        for i, w in enumerate(move):
                            out.append(
                                {
                                    "debug": inst.get("debug", 0),
                                    "engine": inst["engine"],
                                    "ins": [],
                                    "outs": [],
                                    "name": f"{inst['name']}-ws{i}",
                                    "opcode": "NoOp",
                                    "sync_info": {"on_update": [], "on_wait": [w]},
                                }
                            )
                            n_split += 1
                        si["on_wait"] = keep
                out.append(inst)
            bb["instructions"] = out
    if n_split:
        print(f"tilefix: split {n_split} excess waits onto NoOps")
    return _json.dumps(d).encode()


def _install_compile_hook():
    import concourse.bass_utils as bu
    import concourse.bass2jax as b2j

    if getattr(bu, "_tilefix_wrapped", False):
        return
    orig = bu.compile_bir_kernel

    def wrapped(bir_json, tmpdir, neff_name="file.neff"):
        return orig(_split_waits_json(bir_json), tmpdir, neff_name)

    bu.compile_bir_kernel = wrapped
    b2j.compile_bir_kernel = wrapped
    bu._tilefix_wrapped = True


def _install_ntff_hook_shim():
    """The image's antenv package lacks axon_hooks; provide a stand-in module
    exposing the ctypes-based NTFF profile hook against /opt/axon/libaxon_pjrt.so
    so run_bass_kernel_spmd(trace=True) works."""
    import sys as _sys
    import types

    if "antenv.axon_hooks" in _sys.modules:
        return
    mod = types.ModuleType("antenv.axon_hooks")
    _state = {"hook": None}

    so_path = "/opt/axon/libaxon_pjrt.so"
    try:
        import importlib.util

        spec = importlib.util.spec_from_file_location(
            "trn_agent_boot.trn_boot", "/root/.axon_site/trn_agent_boot/trn_boot.py"
        )
        import trn_agent_boot.trn_boot as _tb  # type: ignore

        _state["hook"] = _tb._ntff_profile_via_ctypes(so_path)
    except Exception:
        _state["hook"] = None

    def get_axon_ntff_profile_hook():
        return _state["hook"]

    def set_axon_ntff_profile_hook(h):
        _state["hook"] = h

    mod.get_axon_ntff_profile_hook = get_axon_ntff_profile_hook
    mod.set_axon_ntff_profile_hook = set_axon_ntff_profile_hook
    _sys.modules["antenv.axon_hooks"] = mod


apply()

from concourse import mybir
from concourse.bass_utils import run_bass_kernel_spmd
from concourse.masks import make_identity

F32 = mybir.dt.float32
F32R = mybir.dt.float32r
BF16 = mybir.dt.bfloat16
FP16 = mybir.dt.float16
F8 = mybir.dt.float8e4
EXP = mybir.ActivationFunctionType.Exp
DR = mybir.MatmulPerfMode.DoubleRow

S = 4096          # sequence length
D = 1024          # model dim
NH = 16           # heads
HD = 64           # head dim
NCORES = 8
HPC = NH // NCORES  # heads per core = 2
QT = 512          # qpos tile (free dim of S^T / PV matmuls)
KC = 128          # kpos chunk (partition dim of S^T tiles)
NQT = S // QT     # 8
NKC = S // KC     # 32
NPAIR = NKC // 2  # 16 key-chunk pairs
NEG = -1.0e30

USE_FP8_PV = False       # fp8 PV measured at 2.5e-2 full-output rel err
                         # (gate is 2e-2) - bf16 PV gives 1.7e-3
DIRECT_PSUM_DMA = False  # bass dma_start cannot read PSUM
MASK_ON_POOL = True      # causal zeroing via gpsimd affine_select on pt
                         # (post-exp, SBUF) instead of DVE -1e30 adds on PSUM
DEFER_TAIL = False       # issue step qt's normalize tail inside step qt+1's
                         # pair stream (False = at the end of step qt);
                         # True raced: outproj drains could read attnT
                         # before the deferred tail wrote it
DVE_EXP_FRAC = False     # offload ~1/4 of off-diagonal exps to the DVE via
                         # the bf16 bit-trick; measured 312us vs 292us off -
                         # the DVE FIFO becomes the pacer, keep disabled
# bf16-bit Schraudolph constants: bits = x*0.125*(128/ln2) + 127*128
EXPA = 0.125 * 128.0 / float(np.log(2.0))
EXPB = 127.0 * 128.0 - 2.0 * 128.0 / float(np.log(2.0))
PV_ROWSPLIT = False      # split PV contraction into two 64-row-group matmuls
                         # (tile_position rows 0/64) that run concurrently;
                         # concurrent accumulation into the same PSUM
                         # addresses - suspected runtime hang, keep off
# NOTE: this walrus rejects every bass_isa extended op (gpsimd
# partition_broadcast, custom-DVE reciprocal_approx_*) with "ISA wrong
# length", so the tail keeps the plain DVE reciprocal + f32r ones-matmul
# broadcast; the two heads share one PSUM pb tile via column groups.

_CACHE = {}


def _build_nc():
    nc = bass.Bass("TRN2")

    xT_d = nc.dram_tensor("xT", [D, S], BF16, kind="ExternalInput")
    wq_d = nc.dram_tensor("wq", [D, 128], BF16, kind="ExternalInput")
    wk_d = nc.dram_tensor("wk", [D, 128], BF16, kind="ExternalInput")
    wv_d = nc.dram_tensor("wv", [D, 128], BF16, kind="ExternalInput")
    wo_d = nc.dram_tensor("wo", [128, D], BF16, kind="ExternalInput")
    cos_d = nc.dram_tensor("cosP", [128, S], FP16, kind="ExternalInput")
    sin_d = nc.dram_tensor("sinPs", [128, S], FP16, kind="ExternalInput")
    swp_d = nc.dram_tensor("swapP", [128, 128], BF16, kind="ExternalInput")
    neg_d = nc.dram_tensor("negm", [128, 256], F32, kind="ExternalInput")
    out_d = nc.dram_tensor("out", [S, D], F32, kind="ExternalOutput")

    VDT = F8 if USE_FP8_PV else BF16

    with tile.TileContext(nc) as tc:
        with (
            tc.tile_pool(name="const", bufs=1) as cpool,
            tc.tile_pool(name="big", bufs=1) as big,
            tc.tile_pool(name="xp", bufs=3) as xp,
            tc.tile_pool(name="wk", bufs=6) as wkp,
            tc.tile_pool(name="pt", bufs=8) as ptp,
            tc.tile_pool(name="rc", bufs=4) as rcp_pool,
            tc.tile_pool(name="ob", bufs=4) as obp,
            tc.tile_pool(name="pa", bufs=2, space="PSUM") as pa,
            tc.tile_pool(name="pss", bufs=2, space="PSUM") as pss,
            tc.tile_pool(name="pvh", bufs=2, space="PSUM") as pvh,
        ):
            # ---- minimal consts for proj(0): its matmuls wait only on the
            # projection weights + x chunk DMAs, nothing else ---------------
            identb = cpool.tile([128, 128], BF16)
            make_identity(nc, identb[:])
            w_sb = {}
            for name, dram in (("q", wq_d), ("k", wk_d), ("v", wv_d)):
                t = cpool.tile([128, 8, 128], BF16, name=f"w_{name}")
                nc.sync.dma_start(
                    t[:], dram.ap().rearrange("(kc p) m -> p kc m", p=128)
                )
                w_sb[name] = t
            swap_sb = cpool.tile([128, 128], BF16)
            nc.sync.dma_start(swap_sb[:], swp_d.ap())
            cosP = cpool.tile([128, S], FP16)
            sinPs = cpool.tile([128, S], FP16)
            if not MASK_ON_POOL:
                negm = cpool.tile([128, 256], F32)
            ones1 = cpool.tile([1, 64], F32R)
            expbias = cpool.tile([128, 1], F32)
            wo_sb = cpool.tile([128, D], BF16)

            def late_consts():
                # chunked so step-0 rope only waits on its own slice
                for c8 in range(8):
                    csl = slice(c8 * QT, (c8 + 1) * QT)
                    nc.sync.dma_start(cosP[:, csl], cos_d.ap()[:, csl])
                    nc.sync.dma_start(sinPs[:, csl], sin_d.ap()[:, csl])
                if not MASK_ON_POOL:
                    nc.sync.dma_start(negm[:], neg_d.ap())
                # -1 row: the Newton reciprocal lands as -1/denom, the
                # broadcast matmul against -1 restores the sign
                nc.scalar.activation(
                    ones1[:], cosP[0:1, 0:64],
                    mybir.ActivationFunctionType.Copy, bias=-1.0, scale=0.0,
                )
                nc.vector.memset(expbias[:], -2.0)
                nc.sync.dma_start(wo_sb[:], wo_d.ap())

            # ---- persistent activations -----------------------------------
            qT = big.tile([128, S], BF16)   # rope'd q, [2*64 dims, seq]
            kT = big.tile([128, S], BF16)
            attnT = big.tile([128, S], BF16)  # normalized attn out, [dims, seq]
            # augmented V: [kpos, pair, slot, head, 64 dims + ones col]
            vaug = big.tile([128, NPAIR, 2, HPC, 65], VDT, name="vaug")

            def init_vaug():
                va_flat = vaug[:].rearrange("p a b c o -> p (a b c) o")
                nc.scalar.activation(
                    va_flat[:, :, 64:65], identb[:, 0:NPAIR * 2 * HPC],
                    mybir.ActivationFunctionType.Copy, bias=1.0, scale=0.0,
                )

            xblk = {}

            # ---- projection units (one step = dma + q + k + v) ------------
            def dma_unit(st):
                sl = slice(st * QT, (st + 1) * QT)
                xb = xp.tile([128, 8, QT], BF16, tag="xblk", name=f"xb{st}")
                for kc8 in range(8):
                    nc.sync.dma_start(
                        xb[:, kc8, :],
                        xT_d.ap()[kc8 * 128:(kc8 + 1) * 128, sl],
                    )
                xblk[st] = xb

            def qk_unit(st, which):
                sl = slice(st * QT, (st + 1) * QT)
                dst = qT if which == "q" else kT
                xb = xblk[st]
                pa_t = pa.tile([128, QT], F32, tag="pa")
                for kc8 in range(8):
                    nc.tensor.matmul(
                        pa_t[:], w_sb[which][:, kc8, :], xb[:, kc8, :],
                        start=(kc8 == 0), stop=(kc8 == 7),
                    )
                qsb = wkp.tile([128, QT], BF16, tag="qsb")
                nc.vector.tensor_copy(qsb[:], pa_t[:])
                ps2 = pa.tile([128, QT], F32, tag="pa")
                nc.tensor.matmul(ps2[:], swap_sb[:], qsb[:],
                                 start=True, stop=True)
                t1 = wkp.tile([128, QT], BF16, tag="t1")
                nc.vector.tensor_mul(t1[:], ps2[:], sinPs[:, sl])
                t2 = wkp.tile([128, QT], BF16, tag="t2")
                nc.vector.tensor_mul(t2[:], qsb[:], cosP[:, sl])
                nc.vector.tensor_add(dst[:, sl], t1[:], t2[:])

            def v_unit(st):
                xb = xblk[st]
                pa_t = pa.tile([128, QT], F32, tag="pa")
                for kc8 in range(8):
                    nc.tensor.matmul(
                        pa_t[:], w_sb["v"][:, kc8, :], xb[:, kc8, :],
                        start=(kc8 == 0), stop=(kc8 == 7),
                    )
                vtmp = wkp.tile([128, QT], BF16, tag="vtmp")
                nc.vector.tensor_copy(vtmp[:], pa_t[:])
                for z in range(4):
                    kc = st * 4 + z
                    pr, sl8 = kc // 2, kc % 2
                    pst = pa.tile([128, 128], BF16, tag="pa")
                    nc.tensor.transpose(
                        pst[:], vtmp[:, z * 128:(z + 1) * 128], identb[:]
                    )
                    pst3 = pst[:].rearrange("p (h d) -> p h d", h=HPC)
                    nc.vector.tensor_copy(vaug[:, pr, sl8, :, 0:64], pst3)

            def proj_units(st):
                return [
                    lambda st=st: dma_unit(st),
                    lambda st=st: qk_unit(st, "q"),
                    lambda st=st: qk_unit(st, "k"),
                    lambda st=st: v_unit(st),
                ]

            # ---- attention -------------------------------------------------
            def make_po(qt):
                return [
                    pvh.tile([128, QT], F32, tag="pvh", name=f"po{qt}_{h}")
                    for h in range(HPC)
                ]

            def pair_scores_exp(qt, p, po_h):
                """Scores (2 chunks x 2 heads) + causal mask + exp -> pt tiles.
                Returns the two pt tiles (one per head) for the deferred PV.

                The two heads' score matmuls are issued adjacently: h0's
                stationary/moving operands sit at partitions 0-63 and h1's at
                64-127, so they land in disjoint PE row-groups (auto
                tile_position) and execute concurrently."""
                pairjoff = max(0, (2 * p - 4 * qt)) * 128
                ps_ts = [
                    pss.tile([128, 2, QT], F32, tag="pss",
                             name=f"ss{qt}_{p}_{h}")
                    for h in range(HPC)
                ]
                for s8 in range(2):
                    kc = 2 * p + s8
                    for h in range(HPC):
                        hsl = slice(h * 64, (h + 1) * 64)
                        # write from pairjoff so the pair-wide exp below
                        # never reads uninitialized PSUM (slot 1's columns
                        # below its own causal offset are computed but then
                        # excluded from PV, which reads from joff onward)
                        nc.tensor.matmul(
                            ps_ts[h][:, s8, pairjoff:QT],
                            kT[hsl, kc * 128:(kc + 1) * 128],
                            qT[hsl, qt * QT + pairjoff:(qt + 1) * QT],
                            start=True, stop=True,
                        )
                pts = []
                for h in range(HPC):
                    ps_t = ps_ts[h]
                    diag = p >= 2 * qt
                    if diag and not MASK_ON_POOL:
                        if p == 2 * qt:
                            nc.vector.tensor_add(
                                ps_t[:, 0, 0:128], ps_t[:, 0, 0:128],
                                negm[:, 128:256])
                            nc.vector.tensor_add(
                                ps_t[:, 1, 0:256], ps_t[:, 1, 0:256],
                                negm[:, 0:256])
                        else:
                            nc.vector.tensor_add(
                                ps_t[:, 0, 256:384], ps_t[:, 0, 256:384],
                                negm[:, 128:256])
                            nc.vector.tensor_add(
                                ps_t[:, 1, 256:512], ps_t[:, 1, 256:512],
                                negm[:, 0:256])
                    pt_t = ptp.tile([128, 2, QT], VDT, tag="pt",
                                    name=f"pt{qt}_{p}_{h}")
                    # bias shifts all probs by e^-2 (cancels in the rowsum
                    # normalization) so fp8's max never overflows on tail
                    # scores; fp8 is floating point, so relative resolution
                    # of the probs is unchanged
                    if DVE_EXP_FRAC and not diag and (p % 4 == 1):
                        # bit-trick exp on the DVE for ~1/4 of the
                        # off-diagonal pairs: bf16(exp(x)) bits equal
                        # round(x*128/ln2 + 127*128) in int16 up to the
                        # linear-mantissa sawtooth (<6% per prob, cancels
                        # to ~1e-3 on the output through the softmax);
                        # offloads the saturated scalar engine
                        nc.vector.tensor_scalar(
                            out=pt_t[:, :, pairjoff:QT].bitcast(
                                mybir.dt.int16),
                            in0=ps_t[:, :, pairjoff:QT],
                            scalar1=EXPA, scalar2=EXPB,
                            op0=mybir.AluOpType.mult,
                            op1=mybir.AluOpType.add)
                    else:
                        nc.scalar.activation(
                            pt_t[:, :, pairjoff:QT], ps_t[:, :, pairjoff:QT],
                            EXP, scale=0.125, bias=expbias[:],
                        )
                    if diag and MASK_ON_POOL:
                        # zero the 128-wide causal triangle of each chunk:
                        # keep qpos-col c >= kpos-row p within the window
                        # starting at the chunk's causal offset
                        for s8 in range(2):
                            joff = max(0, 2 * p + s8 - 4 * qt) * 128
                            nc.gpsimd.affine_select(
                                out=pt_t[:, s8, joff:joff + 128],
                                in_=pt_t[:, s8, joff:joff + 128],
                                compare_op=mybir.AluOpType.is_ge,
                                fill=0.0,
                                base=0,
                                pattern=[[1, 128]],
                                channel_multiplier=-1,
                            )
                    pts.append(pt_t)
                return pts

            def pair_pv(qt, p, po_h, pts):
                pairjoff = max(0, (2 * p - 4 * qt)) * 128
                last = (p == 2 * qt + 1)
                for h in range(HPC):
                    if USE_FP8_PV:
                        nc.tensor.matmul(
                            po_h[h][0:65, pairjoff:QT],
                            vaug[:, p, :, h, :],
                            pts[h][:, :, pairjoff:QT],
                            start=(p == 0), stop=last,
                            perf_mode=DR,
                        )
                    elif PV_ROWSPLIT:
                        # split the 128-deep contraction into two 64-row
                        # matmuls pinned to disjoint PE row-groups; they
                        # execute concurrently and accumulate into the same
                        # PSUM tile (drain order is pc order, so the
                        # start=True clear lands first)
                        for s8 in range(2):
                            kc = 2 * p + s8
                            joff = max(0, kc - 4 * qt) * 128
                            for rg in range(2):
                                nc.tensor.matmul(
                                    po_h[h][0:65, joff:QT],
                                    vaug[rg * 64:(rg + 1) * 64, p, s8, h, :],
                                    pts[h][rg * 64:(rg + 1) * 64, s8, joff:QT],
                                    start=(p == 0 and s8 == 0 and rg == 0),
                                    stop=(last and s8 == 1 and rg == 1),
                                    tile_position=(rg * 64, 0),
                                )
                    else:
                        for s8 in range(2):
                            kc = 2 * p + s8
                            joff = max(0, kc - 4 * qt) * 128
                            nc.tensor.matmul(
                                po_h[h][0:65, joff:QT],
                                vaug[:, p, s8, h, :],
                                pts[h][:, s8, joff:QT],
                                start=(p == 0 and s8 == 0), stop=(last and s8 == 1),
                            )

            C_RECIP = float(0x7EF311C3)

            def tail(qt, po_h):
                """Normalize step qt into attnT: -1/denominator via a
                bit-trick seed + one Newton pass (3 standard DVE ops, ~2x
                cheaper than the native iterative-divide RECIPROCAL and
                usable because this walrus rejects all custom-DVE ops),
                then the f32r ones(-1)-broadcast matmul and the normalize
                muls, pipelined per head (walrus rejects a dst-partition-64
                col-tiled matmul, so each head gets its own pb tile)."""
                qsl = slice(qt * QT, (qt + 1) * QT)
                for h in range(HPC):
                    pb = pa.tile([128, QT], F32, tag="pa", name=f"pb{qt}_{h}")
                    # PSUM reads are value-converted per the instruction
                    # dtype, so the bit trick must run on an SBUF copy
                    den = rcp_pool.tile([1, QT], F32, tag="rc",
                                        name=f"dn{qt}_{h}")
                    nc.vector.tensor_copy(den[:], po_h[h][64:65, :])
                    seed = rcp_pool.tile([1, QT], mybir.dt.int32, tag="rc",
                                         name=f"sd{qt}_{h}")
                    nc.vector.tensor_scalar(
                        out=seed[:], in0=den[:].bitcast(mybir.dt.int32),
                        scalar1=C_RECIP, scalar2=-1.0,
                        op0=mybir.AluOpType.subtract,
                        op1=mybir.AluOpType.mult)
                    t_nr = rcp_pool.tile([1, QT], F32, tag="rc",
                                         name=f"tn{qt}_{h}")
                    nc.vector.tensor_mul(t_nr[:], den[:], seed[:].bitcast(F32))
                    rc_t = rcp_pool.tile([1, QT], F32R, tag="rc",
                                         name=f"rc{qt}_{h}")
                    with nc.allow_low_precision(
                        reason="reciprocal at ~11 bits after one Newton "
                               "pass; probs are bf16 anyway"
                    ):
                        nc.vector.scalar_tensor_tensor(
                            out=rc_t[:], in0=t_nr[:], scalar=-2.0,
                            in1=seed[:].bitcast(F32),
                            op0=mybir.AluOpType.add,
                            op1=mybir.AluOpType.mult)
                    nc.tensor.matmul(pb[0:64, :], ones1[:],
                                     rc_t[:], start=True, stop=True)
                    # walrus allows at most one PSUM operand per tensor
                    # op, so bounce the broadcast through SBUF
                    rb = wkp.tile([64, QT], F32, tag="rb", name=f"rb{qt}_{h}")
                    nc.vector.tensor_copy(rb[:], pb[0:64, :])
                    hsl = slice(h * 64, (h + 1) * 64)
                    # 128-col chunks: each outproj unit reads a 128-col
                    # slice of attnT, so it can start as soon as its chunk
                    # of the normalize lands
                    for zc in range(4):
                        csl = slice(qt * QT + zc * 128, qt * QT + (zc + 1) * 128)
                        nc.vector.tensor_mul(
                            attnT[hsl, csl], po_h[h][0:64, zc * 128:(zc + 1) * 128],
                            rb[:, zc * 128:(zc + 1) * 128]
                        )

            def outproj_unit(qt, z, ncol):
                csl = slice(qt * QT + z * 128, qt * QT + (z + 1) * 128)
                osl = slice(ncol * 512, (ncol + 1) * 512)
                pout = pa.tile([128, QT], F32, tag="pa")
                nc.tensor.matmul(pout[:], attnT[:, csl], wo_sb[:, osl],
                                 start=True, stop=True)
                osb = obp.tile([128, QT], F32, tag="ob")
                # DVE does all PSUM->SBUF bounces: the scalar engine is the
                # co-bottleneck (exp stream), DVE has headroom once the
                # full-width reciprocals are gone
                nc.vector.tensor_copy(osb[:], pout[:])
                nc.sync.dma_start(out_d.ap()[csl, osl], osb[:])

            # ---- schedule --------------------------------------------------
            from collections import deque

            fill_q = deque()    # (deadline_step, unit_fn): projections
            deferred = deque()  # outproj units from earlier steps

            u_dma, u_q, u_k, u_v = proj_units(0)
            u_dma()
            late_consts()
            init_vaug()
            u_q(); u_k(); u_v()
            for st in range(1, NQT):
                for u in proj_units(st):
                    fill_q.append((st, u))

            pending_tail = None  # (qt, po_h) normalize deferred into next step
            for qt in range(NQT):
                npairs = 2 * qt + 2
                due = []
                while fill_q and fill_q[0][0] <= qt + 1:
                    due.append(fill_q.popleft()[1])
                po_h = make_po(qt)
                pending = None  # (p, pts) awaiting PV
                due_done = 0
                for p in range(npairs):
                    if DEFER_TAIL and pending_tail is not None and p == 1:
                        # issue the previous step's normalize tail here -
                        # after this step's first scores (PE keeps
                        # streaming over the reciprocal latency) but BEFORE
                        # pair_pv(p=0) below, whose PSUM buffers rotate
                        # onto the tiles the tail still reads, and before
                        # any drained outproj that reads the attnT range
                        # this tail writes
                        tail(*pending_tail)
                        pending_tail = None
                    target = -(-len(due) * (p + 1) // npairs)  # ceil
                    had_due = due_done < target
                    while due_done < target:
                        due[due_done]()
                        due_done += 1
                    # drain deferred outprojs when this pair brought no proj
                    # work; also drain at the first two pairs of each step to
                    # keep PE duty high through the normalize-tail boundary
                    # (the HAM re-throttles on window duty, not just full idle)
                    if deferred and (not had_due or p <= 1):
                        n_drain = 3 if qt >= 4 else (2 if qt >= 2 else 1)
                        if p <= 1:
                            n_drain = max(n_drain, 2)
                        for _ in range(n_drain):
                            if deferred:
                                deferred.popleft()()
                    if pending is not None:
                        pair_pv(qt, pending[0], po_h, pending[1])
                    pending = (p, pair_scores_exp(qt, p, po_h))
                pair_pv(qt, pending[0], po_h, pending[1])
                if DEFER_TAIL:
                    assert pending_tail is None
                    pending_tail = (qt, po_h)
                else:
                    tail(qt, po_h)
                for z in range(4):
                    for ncol in range(2):
                        deferred.append(
                            lambda qt=qt, z=z, ncol=ncol: outproj_unit(qt, z, ncol)
                        )
            if pending_tail is not None:
                tail(*pending_tail)
            while deferred:
                deferred.popleft()()

    return nc


def _rope_tables(token_positions):
    """cosP/sinPs in the transposed per-partition layout.

    Row r (r in 0..127): head = r//64, idx = r%64; pair j = idx%32.
    Rows with idx<32 hold even rope dims (d=2j), idx>=32 odd dims (d=2j+1).
    sinPs is the *swap-adjusted* sin table: the swapped projection holds the
    partner value (q_odd on even rows, q_even on odd rows), so even rows
    need -sin (r_e = q_e cos - q_o sin) and odd rows +sin (r_o = q_o cos +
    q_e sin).
    """
    pos = token_positions.astype(np.float32)  # [S]
    inv = (1.0 / (10000.0 ** (np.arange(0, HD, 2, dtype=np.float32) / HD)))
    freqs = pos[:, None] * inv[None, :]        # [S, 32]
    cos32 = np.cos(freqs).T.astype(np.float32)  # [32, S]
    sin32 = np.sin(freqs).T.astype(np.float32)
    cosP = np.concatenate([cos32, cos32, cos32, cos32], 0)
    sinPs = np.concatenate([-sin32, sin32, -sin32, sin32], 0)
    return (np.ascontiguousarray(cosP).astype(np.float16),
            np.ascontiguousarray(sinPs).astype(np.float16))


def kernel(x, w_q, w_k, w_v, w_o, token_positions):
    x = np.asarray(x, dtype=np.float32)
    w_q = np.asarray(w_q, dtype=np.float32)
    w_k = np.asarray(w_k, dtype=np.float32)
    w_v = np.asarray(w_v, dtype=np.float32)
    w_o = np.asarray(w_o, dtype=np.float32)
    tp = np.asarray(token_positions).reshape(-1)

    b = x.shape[0]
    assert x.shape == (b, S, D) and b == 1

    xT = np.ascontiguousarray(x[0].T).astype(ml_dtypes.bfloat16)  # [D, S]
    cosP, sinPs = _rope_tables(tp)

    # per-head permutation: evens (0,2,..62) then odds (1,3,..63)
    perm64 = np.concatenate([np.arange(0, HD, 2), np.arange(1, HD, 2)])
    # swap of the 32-blocks within each head: [32:64, 0:32] per 64-block
    swap128 = np.concatenate([
        np.arange(32, 64), np.arange(0, 32),
        np.arange(96, 128), np.arange(64, 96),
    ])
    swapP = np.eye(128, dtype=np.float32)[swap128].astype(ml_dtypes.bfloat16)
    negm = np.where(
        np.arange(256)[None, :] < 128 + np.arange(128)[:, None], NEG, 0.0
    ).astype(np.float32)

    if "nc" not in _CACHE:
        _CACHE["nc"] = _build_nc()
    nc = _CACHE["nc"]

    in_maps = []
    for c in range(NCORES):
        rows = np.concatenate(
            [c * 128 + h * 64 + perm64 for h in range(HPC)]
        )  # 128 permuted q/k output dims of this core
        in_maps.append({
            "xT": xT,
            "wq": np.ascontiguousarray(w_q[rows].T).astype(ml_dtypes.bfloat16),
            "wk": np.ascontiguousarray(w_k[rows].T).astype(ml_dtypes.bfloat16),
            "wv": np.ascontiguousarray(
                w_v[c * 128:(c + 1) * 128].T).astype(ml_dtypes.bfloat16),
            "wo": np.ascontiguousarray(
                w_o[:, c * 128:(c + 1) * 128].T).astype(ml_dtypes.bfloat16),
            "cosP": cosP,
            "sinPs": sinPs,
            "swapP": swapP,
            "negm": negm,
        })

    _CACHE["last_in_maps"] = in_maps
    res = run_bass_kernel_spmd(nc, in_maps, core_ids=list(range(NCORES)))
    _CACHE["last_res"] = res
    out = res.results[0]["out"].astype(np.float64)
    for c in range(1, NCORES):
        out += res.results[c]["out"]
    return out.astype(np.float32)[None]


if __name__ == "__main__":
    rng = np.random.default_rng(0)
    x = rng.standard_normal((1, S, D), dtype=np.float32)
    sc = 1.0 / np.sqrt(D)
    wq = rng.standard_normal((D, D), dtype=np.float32) * sc
    wk = rng.standard_normal((D, D), dtype=np.float32) * sc
    wv = rng.standard_normal((D, D), dtype=np.float32) * sc
    wo = rng.standard_normal((D, D), dtype=np.float32) * sc
    tpos = np.arange(S, dtype=np.int32)[None]
    out = kernel(x=x, w_q=wq, w_k=wk, w_v=wv, w_o=wo, token_positions=tpos)
    print("kernel out:", out.shape, out.dtype, float(np.abs(out).max()))
    if "last_res" in _CACHE:
        dbg = _CACHE["last_res"].results[0].get("dbg")
        if dbg is not None:
            names = ["den ", "seed", "t_nr", "rc  "]
            for r in range(4):
                row = dbg[r]
                print(f"dbg {names[r]}: min={np.nanmin(row):.4e} "
                      f"max={np.nanmax(row):.4e} nan={np.isnan(row).sum()}"
                      f" samples={row[[0, 5, 600, 2000, 4095]]}")
        # numpy reference for the per-core partial to localize corruption
        def np_partial_unused(c):
            rows_v = slice(c * 128, (c + 1) * 128)
            qh = x[0] @ w_q.T
            kh = x[0] @ w_k.T
            vh = (x[0] @ w_v.T)[:, rows_v]
            pos = np.arange(S, dtype=np.float32)
            inv = 1.0 / (10000.0 ** (np.arange(0, HD, 2, np.float32) / HD))
            fr = pos[:, None] * inv[None, :]
            cosn, sinn = np.cos(fr), np.sin(fr)
            def rope_np(t):
                t = t.reshape(S, NH, HD)
                e, o = t[:, :, 0::2], t[:, :, 1::2]
                re = e * cosn[:, None, :] - o * sinn[:, None, :]
                ro = e * sinn[:, None, :] + o * cosn[:, None, :]
                z = np.empty_like(t)
                z[:, :, 0::2] = re
                z[:, :, 1::2] = ro
                return z
            qr, kr = rope_np(qh), rope_np(kh)
            msk = np.tril(np.ones((S, S), bool))
            outp = np.zeros((S, D), np.float32)
            for hh in range(2 * c, 2 * c + 2):
                s = qr[:, hh] @ kr[:, hh].T / 8.0
                s = np.where(msk, s, -np.inf)
                p = np.exp(s - s.max(-1, keepdims=True))
                p /= p.sum(-1, keepdims=True)
                at = p @ vh[:, (hh - 2 * c) * 64:(hh - 2 * c + 1) * 64]
                outp += at @ w_o[:, hh * 64:(hh + 1) * 64].T
            return outp
    # race detector: rerun with cached NEFF, compare
    outs = [out]
    for rep in range(3):
        o2 = kernel(x=x, w_q=wq, w_k=wk, w_v=wv, w_o=wo, token_positions=tpos)
        d = float(np.nanmax(np.abs(o2 - outs[0])))
        print(f"rerun {rep}: max={float(np.abs(o2).max()):.4g} "
              f"nan={int(np.isnan(o2).sum())} diff-vs-run0={d:.4g}")
        outs.append(o2)
